# revision 17
# baseline (speedup 1.0000x reference)
"""Axial (frame-local) multi-head attention for Trainium2, 8-core SPMD.

Problem: x:[2,8192,512] -> qkv proj -> per-(batch,head,frame) attention over
n=1024 tokens -> out proj. B=2, f=8 frames, h=8 heads, d=64.

Sharding: the 16 (batch, frame) pairs are embarrassingly parallel; each of
the 8 cores handles 2 pairs end-to-end (weights replicated). Host
pre-transposes x so every on-chip matmul operand is naturally laid out; no
on-chip transposes.

Per-core pipeline (pair p, head hh):
  qkT = [Wq*scale | Wk]^T @ xT      [1024ch, 1024tok]  (ch-major, f32r in,
  v   = xT^T @ Wv                   [1024tok, 512]      bf16 out)
  simT = kT_h^T @ qT_h              [j, i]  bf16 ops, fp32 PSUM. |sim|~1 so
  expT = exp(simT)                  softmax needs no max subtraction.
  oT65 = [v_h | 1]^T @ expT         [65, i] ones col fuses the denominator.
  (after all heads) batched reciprocal of the 16 denominator rows, PE
  ones-matmul broadcasts each across 64 partitions, DVE multiply.
  y    = oTn^T @ Wout + b_out       (bias via K=1 ones matmul into PSUM)
"""
import json
import numpy as np
from contextlib import ExitStack

import concourse.bass as bass
import concourse.tile as tile
import concourse.mybir as mybir
from concourse.bass_utils import run_bass_kernel_spmd

F32 = mybir.dt.float32
F32R = mybir.dt.float32r
BF16 = mybir.dt.bfloat16
AF = mybir.ActivationFunctionType

B, NTOT, DIM = 2, 8192, 512
H, D, F = 8, 64, 8
N = NTOT // F            # 1024 tokens per frame
SCALE = D ** -0.5
NP = 2                   # (batch, frame) pairs per core
TOK = NP * N             # 2048 tokens per core

DT_ATT = BF16            # dtype of attention matmul operands


def _legalize_waits(bir: bytes) -> bytes:
    """TRN2 instructions carry a single HW wait slot and this walrus build
    refuses to split multi-wait instructions; hoist extra waits onto NoOps
    inserted just before, on the same engine stream."""
    j = json.loads(bir)
    ctr = 0
    for fn in j["functions"]:
        for blk in fn["blocks"]:
            out = []
            for inst in blk["instructions"]:
                si = inst.get("sync_info")
                if si:
                    waits = si.get("on_wait") or []
                    if len(waits) > 1:
                        for w in waits[:-1]:
                            ctr += 1
                            nop = {
                                "engine": inst["engine"],
                                "ins": [], "outs": [],
                                "name": f"I-waitfix-{ctr}",
                                "opcode": "NoOp",
                                "sync_info": {"on_update": [], "on_wait": [w]},
                            }
                            if "debug" in inst:
                                nop["debug"] = inst["debug"]
                            out.append(nop)
                        si["on_wait"] = waits[-1:]
                out.append(inst)
            blk["instructions"] = out
    return json.dumps(j).encode()


def build(with_bias=True):
    nc = bass.Bass(trn_type="TRN2")
    xt = nc.dram_tensor("xt", [DIM, TOK], F32R, kind="ExternalInput")
    wqk = nc.dram_tensor("wqk", [DIM, 1024], F32R, kind="ExternalInput")
    wv = nc.dram_tensor("wv", [DIM, 512], F32R, kind="ExternalInput")
    wout = nc.dram_tensor("wout", [DIM, 512], F32R, kind="ExternalInput")
    bout = nc.dram_tensor("bout", [1, 512], F32R, kind="ExternalInput")
    y = nc.dram_tensor("y", [TOK, DIM], F32, kind="ExternalOutput")
    rscr = [nc.dram_tensor(f"rscr{t}", [128, 512], F32) for t in range(4)]

    with tile.TileContext(nc) as tc, ExitStack() as ctx:
        const = ctx.enter_context(tc.tile_pool(name="const", bufs=1))
        qk_pool = ctx.enter_context(tc.tile_pool(name="qk", bufs=2))
        vv_pool = ctx.enter_context(tc.tile_pool(name="vv", bufs=2))
        et_pool = ctx.enter_context(tc.tile_pool(name="et", bufs=4))
        otn_pool = ctx.enter_context(tc.tile_pool(name="otn", bufs=1))
        den_pool = ctx.enter_context(tc.tile_pool(name="den", bufs=1))
        rd_pool = ctx.enter_context(tc.tile_pool(name="rd", bufs=1))
        y_pool = ctx.enter_context(tc.tile_pool(name="yo", bufs=2))
        den_pool2 = ctx.enter_context(tc.tile_pool(name="dsb", bufs=4))
        ps = ctx.enter_context(tc.tile_pool(name="ps", bufs=1, space="PSUM"))

        # ---- constants / weights ----
        wqk_sb = [const.tile([128, 1024], F32R, tag=f"wqk{k}", name=f"wqk{k}")
                  for k in range(4)]
        wv_sb = [const.tile([128, 512], F32R, tag=f"wv{k}", name=f"wv{k}")
                 for k in range(4)]
        wout_sb = [const.tile([128, 512], F32R, tag=f"wout{k}", name=f"wout{k}")
                   for k in range(4)]
        xt_sb = [const.tile([128, TOK], F32R, tag=f"xt{k}", name=f"xt{k}")
                 for k in range(4)]
        bout_sb = const.tile([1, 512], F32R, tag="bout", name="bout")
        for k in range(4):
            nc.sync.dma_start(wqk_sb[k][:, 0:512],
                              wqk.ap()[k * 128:(k + 1) * 128, 0:512])
        for k in range(4):
            nc.sync.dma_start(xt_sb[k][:, 0:512],
                              xt.ap()[k * 128:(k + 1) * 128, 0:512])
        for k in range(4):
            nc.sync.dma_start(wqk_sb[k][:, 512:1024],
                              wqk.ap()[k * 128:(k + 1) * 128, 512:1024])
        for k in range(4):
            nc.sync.dma_start(xt_sb[k][:, 512:N],
                              xt.ap()[k * 128:(k + 1) * 128, 512:N])
        for k in range(4):
            nc.sync.dma_start(wv_sb[k][:], wv.ap()[k * 128:(k + 1) * 128, :])
        for k in range(4):
            nc.sync.dma_start(xt_sb[k][:, N:TOK],
                              xt.ap()[k * 128:(k + 1) * 128, N:TOK])
        for k in range(4):
            nc.sync.dma_start(wout_sb[k][:], wout.ap()[k * 128:(k + 1) * 128, :])
        nc.sync.dma_start(bout_sb[:], bout.ap())

        ones_f = const.tile([128, 128], F32, tag="ones_f", name="ones_f")
        nc.gpsimd.memset(ones_f[:], 1.0)
        ones = const.tile([1, 128], F32R, tag="ones", name="ones")
        nc.vector.tensor_copy(ones[:], ones_f[0:1, :])
        ones8_f = const.tile([128, 8], F32, tag="ones8f", name="ones8f")
        nc.gpsimd.memset(ones8_f[:], 1.0)
        ones8 = const.tile([128, 8], DT_ATT, tag="ones8", name="ones8")
        nc.vector.tensor_copy(ones8[:], ones8_f[:])

        for pi in range(NP):
            t0 = pi * N  # token offset of this pair in xt columns

            # ---- stage A: qkT [1024ch, 1024tok], ch-major, bf16 out ----
            qkT = [qk_pool.tile([128, N], DT_ATT, tag=f"qkT{c}", name=f"qkT{c}")
                   for c in range(8)]
            for cht in range(8):
                for nt in range(2):
                    pa = ps.tile([128, 512], F32, tag="pa", name="pa", bufs=4)
                    for kt in range(4):
                        nc.tensor.matmul(
                            pa[:],
                            wqk_sb[kt][:, cht * 128:(cht + 1) * 128],
                            xt_sb[kt][:, t0 + nt * 512:t0 + (nt + 1) * 512],
                            start=(kt == 0), stop=(kt == 3))
                    nc.vector.tensor_copy(qkT[cht][:, nt * 512:(nt + 1) * 512], pa[:])

            # ---- stage A2: v tok-major, packed per-head with ones col ----
            vv = [vv_pool.tile([128, 8 * 65], DT_ATT, tag=f"vv{t}", name=f"vv{t}")
                  for t in range(8)]
            for tt in range(8):
                pv = ps.tile([128, 512], F32, tag="pa", name="pa", bufs=4)
                for kt in range(4):
                    nc.tensor.matmul(
                        pv[:],
                        xt_sb[kt][:, t0 + tt * 128:t0 + (tt + 1) * 128],
                        wv_sb[kt][:],
                        start=(kt == 0), stop=(kt == 3))
                for hh in range(8):
                    nc.vector.tensor_copy(
                        vv[tt][:, hh * 65:hh * 65 + 64],
                        pv[:, hh * 64:(hh + 1) * 64])
                ones_dst = vv[tt][:].rearrange("p (h c) -> p h c", c=65)[:, :, 64:65]
                ones_src = ones8[:].rearrange("p (h c) -> p h c", c=1)
                nc.vector.tensor_copy(ones_dst, ones_src)

            # ---- stage B: attention, two heads (row groups 0/64) at a
            # time; adjacent sims run concurrently on disjoint row groups ----
            otn_u = [otn_pool.tile([128, N], F32, tag=f"otnu{t}", name=f"otnu{t}")
                     for t in range(4)]
            dens4 = [rd_pool.tile([128, 512], F32, tag=f"dens{t}", name=f"dens{t}")
                     for t in range(4)]
            rcp4 = [rd_pool.tile([128, 512], F32, tag=f"rcp{t}", name=f"rcp{t}")
                    for t in range(4)]
            otn = [otn_pool.tile([128, N], F32R, tag=f"otn{t}", name=f"otn{t}")
                   for t in range(4)]
            for hp in range(4):
                qt_tile, kt_tile = hp, 4 + hp
                hA, hB = 2 * hp, 2 * hp + 1
                po = {(h, i): ps.tile([128, 512], F32, tag="pa", name="po", bufs=4)
                      for h in (0, 1) for i in (0, 1)}
                for jt in range(8):
                    psA = ps.tile([128, 1024], F32, tag="psA", name="psA")
                    psB = ps.tile([128, 1024], F32, tag="psB", name="psB")
                    for it in range(2):
                        nc.tensor.matmul(
                            psA[:, it * 512:(it + 1) * 512],
                            qkT[kt_tile][0:64, jt * 128:(jt + 1) * 128],
                            qkT[qt_tile][0:64, it * 512:(it + 1) * 512],
                            start=True, stop=True, tile_position=(0, 0))
                        nc.tensor.matmul(
                            psB[:, it * 512:(it + 1) * 512],
                            qkT[kt_tile][64:128, jt * 128:(jt + 1) * 128],
                            qkT[qt_tile][64:128, it * 512:(it + 1) * 512],
                            start=True, stop=True, tile_position=(64, 0))
                    etA = et_pool.tile([128, 1024], DT_ATT, tag="etA", name="etA")
                    etB = et_pool.tile([128, 1024], DT_ATT, tag="etB", name="etB")
                    nc.scalar.activation(etA[:], psA[:], AF.Exp)
                    nc.scalar.activation(etB[:], psB[:], AF.Exp)
                    for it in range(2):
                        nc.tensor.matmul(
                            po[(0, it)][0:65, :],
                            vv[jt][:, hA * 65:hA * 65 + 65],
                            etA[:, it * 512:(it + 1) * 512],
                            start=(jt == 0), stop=(jt == 7))
                        nc.tensor.matmul(
                            po[(1, it)][0:65, :],
                            vv[jt][:, hB * 65:hB * 65 + 65],
                            etB[:, it * 512:(it + 1) * 512],
                            start=(jt == 0), stop=(jt == 7))
                # evict numerators + denominator rows
                for it in range(2):
                    nc.scalar.copy(
                        otn_u[hp][0:64, it * 512:(it + 1) * 512], po[(0, it)][0:64, :])
                    nc.scalar.copy(
                        otn_u[hp][64:128, it * 512:(it + 1) * 512], po[(1, it)][0:64, :])
                    nc.vector.tensor_copy(dens4[hp][it * 32:it * 32 + 1, :],
                                          po[(0, it)][64:65, :])
                    nc.vector.tensor_copy(dens4[hp][64 + it * 32:64 + it * 32 + 1, :],
                                          po[(1, it)][64:65, :])
                nc.vector.reciprocal(rcp4[hp][:], dens4[hp][:])
                # broadcast each recip row across 64 partitions via a DRAM
                # bounce (step-0 source APs are only legal on the DRAM side);
                # normalization never touches the PE
                nc.sync.dma_start(rscr[hp].ap(), rcp4[hp][:])
                for it in range(2):
                    den_sb = den_pool2.tile([128, 512], F32, tag="dsb",
                                            name="dsb")
                    nc.sync.dma_start(
                        den_sb[0:64, :],
                        rscr[hp].ap()[it * 32:it * 32 + 1, :].broadcast_to([64, 512]))
                    nc.sync.dma_start(
                        den_sb[64:128, :],
                        rscr[hp].ap()[64 + it * 32:64 + it * 32 + 1, :]
                        .broadcast_to([64, 512]))
                    for half in (0, 64):
                        nc.vector.tensor_mul(
                            otn[hp][half:half + 64, it * 512:(it + 1) * 512],
                            otn_u[hp][half:half + 64, it * 512:(it + 1) * 512],
                            den_sb[half:half + 64, :])

            # ---- stage C: out projection + bias ----
            for tt in range(8):
                py = ps.tile([128, 512], F32, tag="pa", name="pa", bufs=4)
                if with_bias:
                    nc.tensor.matmul(py[:], ones[0:1, :], bout_sb[:],
                                     start=True, stop=False, tile_position=(0, 0))
                for kt in range(4):
                    nc.tensor.matmul(
                        py[:],
                        otn[kt][:, tt * 128:(tt + 1) * 128],
                        wout_sb[kt][:],
                        start=(not with_bias and kt == 0), stop=(kt == 3))
                ysb = y_pool.tile([128, 512], F32, tag="ysb", name="ysb")
                nc.scalar.copy(ysb[:], py[:])
                nc.sync.dma_start(
                    y.ap()[pi * N + tt * 128:pi * N + (tt + 1) * 128, :], ysb[:])

    _orig = nc.to_json_bytes
    nc.to_json_bytes = lambda: _legalize_waits(_orig())
    return nc


_NC_CACHE = []
_last_in_maps = None


def kernel(**inputs) -> np.ndarray:
    x = np.ascontiguousarray(np.asarray(inputs["x"], dtype=np.float32))
    W_qkv = np.asarray(inputs["W_qkv"], dtype=np.float32)
    W_out = np.ascontiguousarray(np.asarray(inputs["W_out"], dtype=np.float32))
    b_out = np.ascontiguousarray(np.asarray(inputs["b_out"], dtype=np.float32))
    f = int(np.asarray(inputs["f"]))
    assert f == F and x.shape == (B, NTOT, DIM)

    Wqk = np.ascontiguousarray(
        np.concatenate([W_qkv[:, :512] * SCALE, W_qkv[:, 512:1024]], axis=1))
    Wv = np.ascontiguousarray(W_qkv[:, 1024:1536])
    bo = np.ascontiguousarray(b_out.reshape(1, 512))

    with_bias = bool(np.any(b_out))
    key = with_bias
    if not _NC_CACHE or _NC_CACHE[0][0] != key:
        _NC_CACHE.clear()
        _NC_CACHE.append((key, build(with_bias)))
    nc = _NC_CACHE[0][1]

    in_maps = []
    for core in range(8):
        pairs = (2 * core, 2 * core + 1)
        xT = np.concatenate(
            [x[p // F, (p % F) * N:(p % F + 1) * N, :].T for p in pairs], axis=1)
        in_maps.append({
            "xt": np.ascontiguousarray(xT),
            "wqk": Wqk, "wv": Wv, "wout": W_out, "bout": bo,
        })

    global _last_in_maps
    _last_in_maps = in_maps
    res = run_bass_kernel_spmd(nc, in_maps, list(range(8)))

    out = np.zeros((B, NTOT, DIM), dtype=np.float32)
    for core in range(8):
        yc = res.results[core]["y"]
        for pi, p in enumerate((2 * core, 2 * core + 1)):
            out[p // F, (p % F) * N:(p % F + 1) * N, :] = yc[pi * N:(pi + 1) * N]
    return out


# revision 18
# speedup vs baseline: 1.2916x; 1.2916x over previous
"""Axial (frame-local) multi-head attention for Trainium2, 8-core SPMD.

Problem: x:[2,8192,512] -> qkv proj -> per-(batch,head,frame) attention over
n=1024 tokens -> out proj. B=2, f=8 frames, h=8 heads, d=64.

Sharding: the 16 (batch, frame) pairs are embarrassingly parallel; each of
the 8 cores handles 2 pairs end-to-end (weights replicated). Host
pre-transposes x so every on-chip matmul operand is naturally laid out; no
on-chip transposes.

Per-core pipeline (pair p, head hh):
  qkT = [Wq*scale | Wk]^T @ xT      [1024ch, 1024tok]  (ch-major, f32r in,
  v   = xT^T @ Wv                   [1024tok, 512]      bf16 out)
  simT = kT_h^T @ qT_h              [j, i]  bf16 ops, fp32 PSUM. |sim|~1 so
  expT = exp(simT)                  softmax needs no max subtraction.
  oT65 = [v_h | 1]^T @ expT         [65, i] ones col fuses the denominator.
  (after all heads) batched reciprocal of the 16 denominator rows, PE
  ones-matmul broadcasts each across 64 partitions, DVE multiply.
  y    = oTn^T @ Wout + b_out       (bias via K=1 ones matmul into PSUM)
"""
import json
import numpy as np
from contextlib import ExitStack

import concourse.bass as bass
import concourse.tile as tile
import concourse.mybir as mybir
from concourse.bass_utils import run_bass_kernel_spmd

F32 = mybir.dt.float32
F32R = mybir.dt.float32r
BF16 = mybir.dt.bfloat16
AF = mybir.ActivationFunctionType

B, NTOT, DIM = 2, 8192, 512
H, D, F = 8, 64, 8
N = NTOT // F            # 1024 tokens per frame
SCALE = D ** -0.5
NP = 2                   # (batch, frame) pairs per core
TOK = NP * N             # 2048 tokens per core

DT_ATT = BF16            # dtype of attention matmul operands


def _legalize_waits(bir: bytes) -> bytes:
    """TRN2 instructions carry a single HW wait slot and this walrus build
    refuses to split multi-wait instructions; hoist extra waits onto NoOps
    inserted just before, on the same engine stream."""
    j = json.loads(bir)
    ctr = 0
    for fn in j["functions"]:
        for blk in fn["blocks"]:
            out = []
            for inst in blk["instructions"]:
                si = inst.get("sync_info")
                if si:
                    waits = si.get("on_wait") or []
                    if len(waits) > 1:
                        for w in waits[:-1]:
                            ctr += 1
                            nop = {
                                "engine": inst["engine"],
                                "ins": [], "outs": [],
                                "name": f"I-waitfix-{ctr}",
                                "opcode": "NoOp",
                                "sync_info": {"on_update": [], "on_wait": [w]},
                            }
                            if "debug" in inst:
                                nop["debug"] = inst["debug"]
                            out.append(nop)
                        si["on_wait"] = waits[-1:]
                out.append(inst)
            blk["instructions"] = out
    return json.dumps(j).encode()


def build(with_bias=True):
    nc = bass.Bass(trn_type="TRN2")
    xt = nc.dram_tensor("xt", [DIM, TOK], F32R, kind="ExternalInput")
    wqk = nc.dram_tensor("wqk", [DIM, 1024], F32R, kind="ExternalInput")
    wv = nc.dram_tensor("wv", [DIM, 512], F32R, kind="ExternalInput")
    wout = nc.dram_tensor("wout", [DIM, 512], F32R, kind="ExternalInput")
    bout = nc.dram_tensor("bout", [1, 512], F32R, kind="ExternalInput")
    y = nc.dram_tensor("y", [TOK, DIM], F32, kind="ExternalOutput")
    rscr = [nc.dram_tensor(f"rscr{t}", [128, 512], F32) for t in range(4)]

    with tile.TileContext(nc) as tc, ExitStack() as ctx:
        const = ctx.enter_context(tc.tile_pool(name="const", bufs=1))
        qk_pool = ctx.enter_context(tc.tile_pool(name="qk", bufs=2))
        vv_pool = ctx.enter_context(tc.tile_pool(name="vv", bufs=2))
        et_pool = ctx.enter_context(tc.tile_pool(name="et", bufs=4))
        otn_pool = ctx.enter_context(tc.tile_pool(name="otn", bufs=1))
        den_pool = ctx.enter_context(tc.tile_pool(name="den", bufs=1))
        rd_pool = ctx.enter_context(tc.tile_pool(name="rd", bufs=1))
        y_pool = ctx.enter_context(tc.tile_pool(name="yo", bufs=2))
        den_pool2 = ctx.enter_context(tc.tile_pool(name="dsb", bufs=4))
        ps = ctx.enter_context(tc.tile_pool(name="ps", bufs=1, space="PSUM"))

        # ---- constants / weights ----
        wqk_sb = [const.tile([128, 1024], F32R, tag=f"wqk{k}", name=f"wqk{k}")
                  for k in range(4)]
        wv_sb = [const.tile([128, 512], F32R, tag=f"wv{k}", name=f"wv{k}")
                 for k in range(4)]
        wout_sb = [const.tile([128, 512], F32R, tag=f"wout{k}", name=f"wout{k}")
                   for k in range(4)]
        xt_sb = [const.tile([128, TOK], F32R, tag=f"xt{k}", name=f"xt{k}")
                 for k in range(4)]
        bout_sb = const.tile([1, 512], F32R, tag="bout", name="bout")
        for k in range(4):
            nc.sync.dma_start(wqk_sb[k][:, 0:512],
                              wqk.ap()[k * 128:(k + 1) * 128, 0:512])
        for k in range(4):
            nc.sync.dma_start(xt_sb[k][:, 0:512],
                              xt.ap()[k * 128:(k + 1) * 128, 0:512])
        for k in range(4):
            nc.sync.dma_start(wqk_sb[k][:, 512:1024],
                              wqk.ap()[k * 128:(k + 1) * 128, 512:1024])
        for k in range(4):
            nc.sync.dma_start(xt_sb[k][:, 512:N],
                              xt.ap()[k * 128:(k + 1) * 128, 512:N])
        for k in range(4):
            nc.sync.dma_start(wv_sb[k][:], wv.ap()[k * 128:(k + 1) * 128, :])
        for k in range(4):
            nc.sync.dma_start(xt_sb[k][:, N:TOK],
                              xt.ap()[k * 128:(k + 1) * 128, N:TOK])
        for k in range(4):
            nc.sync.dma_start(wout_sb[k][:], wout.ap()[k * 128:(k + 1) * 128, :])
        nc.sync.dma_start(bout_sb[:], bout.ap())

        ones_f = const.tile([128, 128], F32, tag="ones_f", name="ones_f")
        nc.gpsimd.memset(ones_f[:], 1.0)
        ones = const.tile([1, 128], F32R, tag="ones", name="ones")
        nc.vector.tensor_copy(ones[:], ones_f[0:1, :])
        ones8_f = const.tile([128, 8], F32, tag="ones8f", name="ones8f")
        nc.gpsimd.memset(ones8_f[:], 1.0)
        ones8 = const.tile([128, 8], DT_ATT, tag="ones8", name="ones8")
        nc.vector.tensor_copy(ones8[:], ones8_f[:])

        for pi in range(NP):
            t0 = pi * N  # token offset of this pair in xt columns

            # ---- stage A: qkT [1024ch, 1024tok], ch-major, bf16 out ----
            qkT = [qk_pool.tile([128, N], DT_ATT, tag=f"qkT{c}", name=f"qkT{c}")
                   for c in range(8)]
            for cht in range(8):
                for nt in range(2):
                    pa = ps.tile([128, 512], F32, tag="pa", name="pa", bufs=4)
                    for kt in range(4):
                        nc.tensor.matmul(
                            pa[:],
                            wqk_sb[kt][:, cht * 128:(cht + 1) * 128],
                            xt_sb[kt][:, t0 + nt * 512:t0 + (nt + 1) * 512],
                            start=(kt == 0), stop=(kt == 3))
                    nc.vector.tensor_copy(qkT[cht][:, nt * 512:(nt + 1) * 512], pa[:])

            # ---- stage A2: v tok-major, packed per-head with ones col ----
            vv = [vv_pool.tile([128, 8 * 65], DT_ATT, tag=f"vv{t}", name=f"vv{t}")
                  for t in range(8)]
            for tt in range(8):
                pv = ps.tile([128, 512], F32, tag="pa", name="pa", bufs=4)
                for kt in range(4):
                    nc.tensor.matmul(
                        pv[:],
                        xt_sb[kt][:, t0 + tt * 128:t0 + (tt + 1) * 128],
                        wv_sb[kt][:],
                        start=(kt == 0), stop=(kt == 3))
                for hh in range(8):
                    nc.vector.tensor_copy(
                        vv[tt][:, hh * 65:hh * 65 + 64],
                        pv[:, hh * 64:(hh + 1) * 64])
                ones_dst = vv[tt][:].rearrange("p (h c) -> p h c", c=65)[:, :, 64:65]
                ones_src = ones8[:].rearrange("p (h c) -> p h c", c=1)
                nc.vector.tensor_copy(ones_dst, ones_src)

            # ---- stage B: attention, two heads (row groups 0/64) at a
            # time; adjacent sims run concurrently on disjoint row groups ----
            otn_u = [otn_pool.tile([128, N], F32, tag=f"otnu{t}", name=f"otnu{t}")
                     for t in range(4)]
            dens4 = [rd_pool.tile([128, 512], F32, tag=f"dens{t}", name=f"dens{t}")
                     for t in range(4)]
            rcp4 = [rd_pool.tile([128, 512], F32, tag=f"rcp{t}", name=f"rcp{t}")
                    for t in range(4)]
            otn = [otn_pool.tile([128, N], F32R, tag=f"otn{t}", name=f"otn{t}")
                   for t in range(4)]
            for hp in range(4):
                qt_tile, kt_tile = hp, 4 + hp
                hA, hB = 2 * hp, 2 * hp + 1
                po = {(h, i): ps.tile([128, 512], F32, tag="pa", name="po", bufs=4)
                      for h in (0, 1) for i in (0, 1)}
                for jt in range(8):
                    psA = ps.tile([128, 1024], F32, tag="psA", name="psA")
                    psB = ps.tile([128, 1024], F32, tag="psB", name="psB")
                    for it in range(2):
                        nc.tensor.matmul(
                            psA[:, it * 512:(it + 1) * 512],
                            qkT[kt_tile][0:64, jt * 128:(jt + 1) * 128],
                            qkT[qt_tile][0:64, it * 512:(it + 1) * 512],
                            start=True, stop=True, tile_position=(0, 0))
                        nc.tensor.matmul(
                            psB[:, it * 512:(it + 1) * 512],
                            qkT[kt_tile][64:128, jt * 128:(jt + 1) * 128],
                            qkT[qt_tile][64:128, it * 512:(it + 1) * 512],
                            start=True, stop=True, tile_position=(64, 0))
                    etA = et_pool.tile([128, 1024], DT_ATT, tag="etA", name="etA")
                    etB = et_pool.tile([128, 1024], DT_ATT, tag="etB", name="etB")
                    nc.scalar.activation(etA[:], psA[:], AF.Exp)
                    nc.scalar.activation(etB[:], psB[:], AF.Exp)
                    for it in range(2):
                        nc.tensor.matmul(
                            po[(0, it)][0:65, :],
                            vv[jt][:, hA * 65:hA * 65 + 65],
                            etA[:, it * 512:(it + 1) * 512],
                            start=(jt == 0), stop=(jt == 7))
                        nc.tensor.matmul(
                            po[(1, it)][0:65, :],
                            vv[jt][:, hB * 65:hB * 65 + 65],
                            etB[:, it * 512:(it + 1) * 512],
                            start=(jt == 0), stop=(jt == 7))
                # evict numerators + denominator rows
                for it in range(2):
                    nc.vector.tensor_copy(
                        otn_u[hp][0:64, it * 512:(it + 1) * 512], po[(0, it)][0:64, :])
                    nc.vector.tensor_copy(
                        otn_u[hp][64:128, it * 512:(it + 1) * 512], po[(1, it)][0:64, :])
                    nc.vector.tensor_copy(dens4[hp][it * 32:it * 32 + 1, :],
                                          po[(0, it)][64:65, :])
                    nc.vector.tensor_copy(dens4[hp][64 + it * 32:64 + it * 32 + 1, :],
                                          po[(1, it)][64:65, :])
                nc.vector.reciprocal(rcp4[hp][:], dens4[hp][:])
                # broadcast each recip row across 64 partitions via a DRAM
                # bounce (step-0 source APs are only legal on the DRAM side);
                # normalization never touches the PE
                nc.sync.dma_start(rscr[hp].ap(), rcp4[hp][:])
                for it in range(2):
                    den_sb = den_pool2.tile([128, 512], F32, tag="dsb",
                                            name="dsb")
                    nc.sync.dma_start(
                        den_sb[0:64, :],
                        rscr[hp].ap()[it * 32:it * 32 + 1, :].broadcast_to([64, 512]))
                    nc.sync.dma_start(
                        den_sb[64:128, :],
                        rscr[hp].ap()[64 + it * 32:64 + it * 32 + 1, :]
                        .broadcast_to([64, 512]))
                    for half in (0, 64):
                        nc.vector.tensor_mul(
                            otn[hp][half:half + 64, it * 512:(it + 1) * 512],
                            otn_u[hp][half:half + 64, it * 512:(it + 1) * 512],
                            den_sb[half:half + 64, :])

            # ---- stage C: out projection + bias ----
            for tt in range(8):
                py = ps.tile([128, 512], F32, tag="pa", name="pa", bufs=4)
                if with_bias:
                    nc.tensor.matmul(py[:], ones[0:1, :], bout_sb[:],
                                     start=True, stop=False, tile_position=(0, 0))
                for kt in range(4):
                    nc.tensor.matmul(
                        py[:],
                        otn[kt][:, tt * 128:(tt + 1) * 128],
                        wout_sb[kt][:],
                        start=(not with_bias and kt == 0), stop=(kt == 3))
                ysb = y_pool.tile([128, 512], F32, tag="ysb", name="ysb")
                nc.vector.tensor_copy(ysb[:], py[:])
                nc.sync.dma_start(
                    y.ap()[pi * N + tt * 128:pi * N + (tt + 1) * 128, :], ysb[:])

    _orig = nc.to_json_bytes
    nc.to_json_bytes = lambda: _legalize_waits(_orig())
    return nc


_NC_CACHE = []
_last_in_maps = None


def kernel(**inputs) -> np.ndarray:
    x = np.ascontiguousarray(np.asarray(inputs["x"], dtype=np.float32))
    W_qkv = np.asarray(inputs["W_qkv"], dtype=np.float32)
    W_out = np.ascontiguousarray(np.asarray(inputs["W_out"], dtype=np.float32))
    b_out = np.ascontiguousarray(np.asarray(inputs["b_out"], dtype=np.float32))
    f = int(np.asarray(inputs["f"]))
    assert f == F and x.shape == (B, NTOT, DIM)

    Wqk = np.ascontiguousarray(
        np.concatenate([W_qkv[:, :512] * SCALE, W_qkv[:, 512:1024]], axis=1))
    Wv = np.ascontiguousarray(W_qkv[:, 1024:1536])
    bo = np.ascontiguousarray(b_out.reshape(1, 512))

    with_bias = bool(np.any(b_out))
    key = with_bias
    if not _NC_CACHE or _NC_CACHE[0][0] != key:
        _NC_CACHE.clear()
        _NC_CACHE.append((key, build(with_bias)))
    nc = _NC_CACHE[0][1]

    in_maps = []
    for core in range(8):
        pairs = (2 * core, 2 * core + 1)
        xT = np.concatenate(
            [x[p // F, (p % F) * N:(p % F + 1) * N, :].T for p in pairs], axis=1)
        in_maps.append({
            "xt": np.ascontiguousarray(xT),
            "wqk": Wqk, "wv": Wv, "wout": W_out, "bout": bo,
        })

    global _last_in_maps
    _last_in_maps = in_maps
    res = run_bass_kernel_spmd(nc, in_maps, list(range(8)))

    out = np.zeros((B, NTOT, DIM), dtype=np.float32)
    for core in range(8):
        yc = res.results[core]["y"]
        for pi, p in enumerate((2 * core, 2 * core + 1)):
            out[p // F, (p % F) * N:(p % F + 1) * N, :] = yc[pi * N:(pi + 1) * N]
    return out
